# revision 25
# baseline (speedup 1.0000x reference)
"""NetVLAD Trainium2 Bass kernel (v4).

Full inputs -> full output. Shards batch N=64 across 8 NeuronCores
(8 samples per core), runs one SPMD Bass/Tile kernel, gathers.

HW-calibrated design (probes showed gpsimd ~1us/op and ~0.5us per
matmul weight reload, both several times the cost model):
  - Host ships TWO bf16 layouts of x: natural [C, P] for mm1 and
    pixel-major xT padded to 1024 pixels (pad pixels = 1.0) for the
    mm2 rhs + ssq input. Same HBM bytes as one f32 layout; no on-chip
    x transposes.
  - PE holds weight reloads to 13/sample: mm1 streams 2x450 bf16
    columns against 4 stationary cwT chunks; 8 logit transposes share
    one identity; mm2's S-term matmul reuses the SAME stationary aT
    as the main matmul (rhs = norm column window, v1's trick).
  - aT is f32r (not bf16) so S = aT.T @ norm stays accurate at the
    graded 1e-3*scale floor; the main matmul mixes f32r lhsT with
    bf16 rhs.
  - ssq split: 4 chunks DVE bn_stats, 4 chunks ACT Square+accum;
    bn-stats combine on DVE (gpsimd is ~1us/op on HW - Pool is used
    for nothing in steady state).
  - invn/norm = exp(-/+0.5*ln(ssq)); Ln/Exp/Copy/Square pinned to the
    single ACT table set containing them all (no set reloads).
  - softmax: one DVE pass scales logT by invn (broadcast AP), ONE
    batched ACT exp, DVE per-chunk sums, aT = e*(invs*invn) as one
    DVE broadcast mul.
  - out = main + (-S)*cen: (-S)*cen on ACT (scale AP), add on DVE.
"""
import numpy as np

N, C, H, W = 64, 512, 30, 30
P = H * W              # 900
PPAD = 1024            # padded pixel count for xT (pad pixels are 1.0)
K = 64
NCORES = 8
S = N // NCORES        # samples per core
CCH = 4                # channel chunks of 128
PCHUNKS = 8            # pixel chunks per sample (padded): 8x128
SSQ_DVE_CHUNKS = 4     # pixel chunks whose ssq runs on DVE bn_stats

_cache = {}


def _build_module(repeat=1):
    import concourse.bacc as bacc
    import concourse.bass as bass
    import concourse.tile as tile
    import concourse.mybir as mybir

    F32 = mybir.dt.float32
    F32R = mybir.dt.float32r
    BF16 = mybir.dt.bfloat16
    AF = mybir.ActivationFunctionType
    AX = mybir.AxisListType

    nc = bacc.Bacc("TRN2", target_bir_lowering=False, debug=False,
                   num_devices=NCORES)

    # Pin the activation functions we use to the one table set containing
    # them all (see v3 note: the load-insertion pass maps each activation
    # to the FIRST set listing its function, which otherwise thrashes).
    import concourse.hw_specs as hw_specs
    _tabs = hw_specs.get_activation_tables(nc.m.arch)
    _target = "natural_log_exp_and_others"
    _orig_get_tables = bacc.get_activation_tables
    if _target in _tabs:
        _pin = {AF.Ln, AF.Exp, AF.Copy, AF.Identity, AF.Square}
        _patched = {
            name: (set(funcs) | _pin if name == _target else set(funcs) - _pin)
            for name, funcs in _tabs.items()
        }
        bacc.get_activation_tables = lambda arch: _patched

    x_d = nc.dram_tensor("x", [S, C, P], BF16, kind="ExternalInput").ap()
    xt_d = nc.dram_tensor("xt", [S, PPAD, C], F32R, kind="ExternalInput").ap()
    cwT_d = nc.dram_tensor("cwT", [C, K], BF16, kind="ExternalInput").ap()
    cen_d = nc.dram_tensor("cen", [K, C], F32, kind="ExternalInput").ap()
    id_d = nc.dram_tensor("ident", [128, 128], BF16, kind="ExternalInput").ap()
    out_d = nc.dram_tensor("vlad", [S, K, C], F32, kind="ExternalOutput").ap()

    with tile.TileContext(nc) as tc:
        with (
            tc.tile_pool(name="consts", bufs=1) as consts,
            tc.tile_pool(name="xnat", bufs=5) as xnat_pool,
            tc.tile_pool(name="xtp", bufs=4) as xt_pool,
            tc.tile_pool(name="work", bufs=4) as work,
            tc.tile_pool(name="stats", bufs=4) as stats_pool,
            tc.tile_pool(name="sqscr", bufs=3) as sqscr_pool,
            tc.tile_pool(name="outsb", bufs=4) as outsb_pool,
            tc.tile_pool(name="pvec", bufs=5) as pvec_pool,
            tc.tile_pool(name="pslogits", bufs=1, space="PSUM") as pslogits,
            tc.tile_pool(name="pslogT", bufs=2, space="PSUM") as pslogT,
            tc.tile_pool(name="psmain", bufs=2, space="PSUM") as psmain,
            tc.tile_pool(name="psS", bufs=2, space="PSUM") as psS,
        ):
            # ---- constants ----
            cwT = consts.tile([128, CCH, K], BF16, tag="cwT")
            nc.sync.dma_start(
                cwT[:], cwT_d.rearrange("(j i) k -> i j k", i=128))
            ident = consts.tile([128, 128], BF16, tag="ident")
            nc.sync.dma_start(ident[:], id_d)
            cen = consts.tile([K, C], F32, tag="cen")
            nc.sync.dma_start(cen[:], cen_d)

            def stage1a(s):
                """DMAs, first ssq chunks, mm1, logits copies."""
                xt = xt_pool.tile([128, PCHUNKS, C], F32R, tag="xt")
                nc.sync.dma_start(
                    xt[:], xt_d[s].rearrange("(pj i) c -> i pj c", i=128))
                xna = xnat_pool.tile([128, CCH, P], BF16, tag="xna")
                nc.scalar.dma_start(
                    xna[:], x_d[s].rearrange("(j i) p -> i j p", i=128))

                nd = SSQ_DVE_CHUNKS
                ssqc = pvec_pool.tile([128, PCHUNKS], F32, tag="ssqc")
                stats = stats_pool.tile([128, nd, 6], F32, tag="stats")
                scr = sqscr_pool.tile([128, C], F32, tag="scr")
                for pj in range(nd // 2):
                    nc.vector.bn_stats(stats[:, pj, :], xt[:, pj, :].bitcast(F32))
                for pj in range(nd, nd + (PCHUNKS - nd) // 2):
                    nc.scalar.activation(
                        scr[:], xt[:, pj, :].bitcast(F32), AF.Square,
                        accum_out=ssqc[:, pj:pj + 1])

                # mm1: logits[K, P] (bf16, 4 stationary cwT chunks)
                logA = pslogits.tile([K, 450], F32, tag="logA")
                logB = pslogits.tile([K, 450], F32, tag="logB")
                for j in range(CCH):
                    nc.tensor.matmul(
                        logA[:], cwT[:, j, :], xna[:, j, 0:450],
                        start=(j == 0), stop=(j == CCH - 1))
                    nc.tensor.matmul(
                        logB[:], cwT[:, j, :], xna[:, j, 450:900],
                        start=(j == 0), stop=(j == CCH - 1))

                # logits -> sbuf bf16 (DVE+ACT halves); pad 900:1024 = 0
                logsb = work.tile([K, PPAD], BF16, tag="logsb")
                nc.vector.memset(logsb[:, P:PPAD], 0.0)
                nc.scalar.copy(logsb[:, 0:450], logA[:])
                nc.scalar.copy(logsb[:, 450:900], logB[:])
                return s, xt, ssqc, stats, scr, logsb

            def stage1b(st):
                """Remaining ssq chunks, combine, logit transposes."""
                s, xt, ssqc, stats, scr, logsb = st
                nd = SSQ_DVE_CHUNKS
                for pj in range(nd // 2, nd):
                    nc.vector.bn_stats(stats[:, pj, :], xt[:, pj, :].bitcast(F32))
                for pj in range(nd + (PCHUNKS - nd) // 2, PCHUNKS):
                    nc.scalar.activation(
                        scr[:], xt[:, pj, :].bitcast(F32), AF.Square,
                        accum_out=ssqc[:, pj:pj + 1])
                # combine bn_stats -> ssq on DVE:
                # ssq = cvar_e + cvar_o + 256*(mean_e^2 + mean_o^2)
                sqm = pvec_pool.tile([128, nd, 2], F32, tag="sqm")
                nc.vector.tensor_mul(
                    sqm[:], stats[:, :, 1:6:3], stats[:, :, 1:6:3])
                vsum = pvec_pool.tile([128, nd], F32, tag="vsum")
                nc.vector.tensor_add(
                    vsum[:], stats[:, :, 2:3], stats[:, :, 5:6])
                msum = pvec_pool.tile([128, nd], F32, tag="msum")
                nc.vector.tensor_add(msum[:], sqm[:, :, 0], sqm[:, :, 1])
                m256 = pvec_pool.tile([128, nd], F32, tag="m256")
                nc.vector.tensor_scalar(
                    m256[:], msum[:], 256.0, None,
                    op0=mybir.AluOpType.mult)
                nc.vector.tensor_add(ssqc[:, 0:nd], vsum[:], m256[:])

                # transpose logits -> logT [pixel, K] (shared identity)
                logT = pslogT.tile([128, PCHUNKS * K], BF16, tag="logT")
                for pj in range(PCHUNKS):
                    nc.tensor.matmul(
                        logT[:, K * pj:K * (pj + 1)],
                        logsb[:, 128 * pj:128 * (pj + 1)],
                        ident[0:K, 0:K],
                        is_transpose=True,
                        skip_group_check=True,
                    )
                return s, xt, ssqc, logT

            def stage2a(st):
                """ln / +-0.5 exps on ACT, prescale on DVE."""
                s, xt, ssqc, logT = st
                lssq = pvec_pool.tile([128, PCHUNKS], F32, tag="lssq")
                nc.scalar.activation(lssq[:], ssqc[:], AF.Ln)
                invn = pvec_pool.tile([128, PCHUNKS], F32, tag="invn")
                nc.scalar.activation(invn[:], lssq[:], AF.Exp, scale=-0.5)
                # norm (padded to 9 cols; col 8 zeroed) for the S matmul rhs
                normc = pvec_pool.tile([128, PCHUNKS + 1], F32R, tag="normc")
                nc.vector.memset(normc[:, PCHUNKS:PCHUNKS + 1].bitcast(F32), 0.0)
                nc.scalar.activation(
                    normc[:, 0:PCHUNKS], lssq[:], AF.Exp, scale=0.5)
                lsc = work.tile([128, PCHUNKS, K], F32, tag="lsc")
                nc.vector.tensor_mul(
                    lsc[:],
                    logT[:].rearrange("i (c k) -> i c k", k=K),
                    invn[:].to_broadcast([128, PCHUNKS, K]))
                return s, xt, invn, normc, lsc

            def stage2b(st):
                """exp, softmax sums, aT."""
                s, xt, invn, normc, lsc = st
                e_sb = work.tile([128, PCHUNKS * K], F32, tag="esb")
                nc.scalar.activation(e_sb[:], lsc[:], AF.Exp)

                scol = pvec_pool.tile([128, PCHUNKS], F32, tag="scol")
                nc.vector.reduce_sum(
                    scol[:], e_sb[:].rearrange("i (c k) -> i c k", k=K),
                    axis=AX.X)
                invs = pvec_pool.tile([128, PCHUNKS], F32, tag="invs")
                nc.vector.reciprocal(invs[:], scol[:])
                tcol = pvec_pool.tile([128, PCHUNKS], F32, tag="tcol")
                nc.vector.tensor_mul(tcol[:], invs[:], invn[:])

                # aT = e * t (one DVE broadcast mul, f32r out for mm2)
                aT = work.tile([128, PCHUNKS, K], F32R, tag="aT")
                nc.vector.tensor_mul(
                    aT[:],
                    e_sb[:].rearrange("i (c k) -> i c k", k=K),
                    tcol[:].to_broadcast([128, PCHUNKS, K]))
                return s, xt, aT, normc

            def stage3(st):
                s, xt, aT, normc = st
                # mm2: main += aT.T @ xt ; S += aT.T @ norm (shared lhsT)
                main_ps = psmain.tile([K, C], F32, tag="main")
                S_ps = psS.tile([K, 2], F32, tag="Sps")
                for pj in range(PCHUNKS):
                    pw = 4 if pj == PCHUNKS - 1 else 128
                    nc.tensor.matmul(
                        main_ps[:], aT[0:pw, pj, :],
                        xt[0:pw, pj, :],
                        start=(pj == 0), stop=(pj == PCHUNKS - 1))
                    nc.tensor.matmul(
                        S_ps[:], aT[0:pw, pj, :],
                        normc[0:pw, pj:pj + 2],
                        start=(pj == 0), stop=(pj == PCHUNKS - 1))

                # final: out = main + (-S)*cen
                negS = pvec_pool.tile([K, 1], F32, tag="negS")
                nc.vector.tensor_scalar(
                    negS[:], S_ps[:, 0:1], -1.0, None,
                    op0=mybir.AluOpType.mult)
                tmp = outsb_pool.tile([K, C], F32, tag="tmp")
                nc.scalar.activation(tmp[:], cen[:], AF.Copy, scale=negS[:])
                out_sb = outsb_pool.tile([K, C], F32, tag="outsb")
                nc.vector.tensor_add(out_sb[:], main_ps[:], tmp[:])
                nc.sync.dma_start(out_d[s], out_sb[:])

            # skewed software pipeline:
            # iter k emits: stage3(k-1), 1a(k), 2a(k-1), 1b(k), 2b(k-1)
            samples = [s for _ in range(repeat) for s in range(S)]
            s1 = s2 = None
            for s in samples:
                a = stage1a(s)
                if s1 is not None:
                    mid = stage2a(s1)
                b = stage1b(a)
                if s1 is not None:
                    s2 = stage2b(mid)
                    stage3(s2)
                s1 = b
            stage3(stage2b(stage2a(s1)))

    try:
        nc.compile()
    finally:
        bacc.get_activation_tables = _orig_get_tables
    return nc


def _get_nc(repeat=1):
    key = ("nc", repeat)
    if key not in _cache:
        _cache[key] = _build_module(repeat)
    return _cache[key]


def build_in_maps(x, conv_w, centroids):
    import ml_dtypes

    bf16 = ml_dtypes.bfloat16
    x = np.asarray(x, dtype=np.float32)
    conv_w = np.asarray(conv_w, dtype=np.float32)
    centroids = np.asarray(centroids, dtype=np.float32)

    xs = x.reshape(N, C, P)
    x_bf = xs.astype(bf16)
    xt_f = np.ones((N, PPAD, C), dtype=np.float32)
    xt_f[:, :P, :] = xs.transpose(0, 2, 1)
    cwT = np.ascontiguousarray(conv_w.T).astype(bf16)   # [C, K]
    ident = np.eye(128, dtype=bf16)

    in_maps = []
    for core in range(NCORES):
        in_maps.append({
            "x": np.ascontiguousarray(x_bf[core * S:(core + 1) * S]),
            "xt": np.ascontiguousarray(xt_f[core * S:(core + 1) * S]),
            "cwT": cwT, "cen": centroids, "ident": ident,
        })
    return in_maps


def kernel(x, conv_w, centroids):
    from concourse.bass_utils import run_bass_kernel_spmd

    nc = _get_nc()
    in_maps = build_in_maps(x, conv_w, centroids)
    res = run_bass_kernel_spmd(nc, in_maps, core_ids=list(range(NCORES)))
    out = np.concatenate([r["vlad"] for r in res.results], axis=0)
    return out.reshape(N, K, C)


# revision 27
# speedup vs baseline: 1.5078x; 1.5078x over previous
"""NetVLAD Trainium2 Bass kernel (v4).

Full inputs -> full output. Shards batch N=64 across 8 NeuronCores
(8 samples per core), runs one SPMD Bass/Tile kernel, gathers.

HW-calibrated design (probes showed gpsimd ~1us/op and ~0.5us per
matmul weight reload, both several times the cost model):
  - Host ships TWO bf16 layouts of x: natural [C, P] for mm1 and
    pixel-major xT padded to 1024 pixels (pad pixels = 1.0) for the
    mm2 rhs + ssq input. Same HBM bytes as one f32 layout; no on-chip
    x transposes.
  - PE holds weight reloads to 13/sample: mm1 streams 2x450 bf16
    columns against 4 stationary cwT chunks; 8 logit transposes share
    one identity; mm2's S-term matmul reuses the SAME stationary aT
    as the main matmul (rhs = norm column window, v1's trick).
  - aT is f32r (not bf16) so S = aT.T @ norm stays accurate at the
    graded 1e-3*scale floor; the main matmul mixes f32r lhsT with
    bf16 rhs.
  - ssq split: 4 chunks DVE bn_stats, 4 chunks ACT Square+accum;
    bn-stats combine on DVE (gpsimd is ~1us/op on HW - Pool is used
    for nothing in steady state).
  - invn/norm = exp(-/+0.5*ln(ssq)); Ln/Exp/Copy/Square pinned to the
    single ACT table set containing them all (no set reloads).
  - softmax: one DVE pass scales logT by invn (broadcast AP), ONE
    batched ACT exp, DVE per-chunk sums, aT = e*(invs*invn) as one
    DVE broadcast mul.
  - out = main + (-S)*cen: (-S)*cen on ACT (scale AP), add on DVE.
"""
import numpy as np

N, C, H, W = 64, 512, 30, 30
P = H * W              # 900
PPAD = 1024            # padded pixel count for xT (pad pixels are 1.0)
K = 64
NCORES = 8
S = N // NCORES        # samples per core
CCH = 4                # channel chunks of 128
PCHUNKS = 8            # pixel chunks per sample (padded): 8x128
SSQ_DVE_CHUNKS = 4     # pixel chunks whose ssq runs on DVE bn_stats

_cache = {}


def _build_module(repeat=1):
    import concourse.bacc as bacc
    import concourse.bass as bass
    import concourse.tile as tile
    import concourse.mybir as mybir

    F32 = mybir.dt.float32
    F32R = mybir.dt.float32r
    BF16 = mybir.dt.bfloat16
    AF = mybir.ActivationFunctionType
    AX = mybir.AxisListType

    nc = bacc.Bacc("TRN2", target_bir_lowering=False, debug=False,
                   num_devices=NCORES)

    # Pin the activation functions we use to the one table set containing
    # them all (see v3 note: the load-insertion pass maps each activation
    # to the FIRST set listing its function, which otherwise thrashes).
    import concourse.hw_specs as hw_specs
    _tabs = hw_specs.get_activation_tables(nc.m.arch)
    _target = "natural_log_exp_and_others"
    _orig_get_tables = bacc.get_activation_tables
    if _target in _tabs:
        _pin = {AF.Ln, AF.Exp, AF.Copy, AF.Identity, AF.Square}
        _patched = {
            name: (set(funcs) | _pin if name == _target else set(funcs) - _pin)
            for name, funcs in _tabs.items()
        }
        bacc.get_activation_tables = lambda arch: _patched

    x_d = nc.dram_tensor("x", [S, 128, CCH, P], BF16, kind="ExternalInput").ap()
    xt_d = nc.dram_tensor("xt", [S, 128, PCHUNKS, C], F32R, kind="ExternalInput").ap()
    cwT_d = nc.dram_tensor("cwT", [C, K], BF16, kind="ExternalInput").ap()
    cen_d = nc.dram_tensor("cen", [K, C], F32, kind="ExternalInput").ap()
    id_d = nc.dram_tensor("ident", [128, 128], BF16, kind="ExternalInput").ap()
    out_d = nc.dram_tensor("vlad", [S, K, C], F32, kind="ExternalOutput").ap()

    with tile.TileContext(nc) as tc:
        with (
            tc.tile_pool(name="consts", bufs=1) as consts,
            tc.tile_pool(name="xnat", bufs=5) as xnat_pool,
            tc.tile_pool(name="xtp", bufs=4) as xt_pool,
            tc.tile_pool(name="work", bufs=4) as work,
            tc.tile_pool(name="stats", bufs=4) as stats_pool,
            tc.tile_pool(name="sqscr", bufs=3) as sqscr_pool,
            tc.tile_pool(name="outsb", bufs=4) as outsb_pool,
            tc.tile_pool(name="pvec", bufs=5) as pvec_pool,
            tc.tile_pool(name="pslogits", bufs=1, space="PSUM") as pslogits,
            tc.tile_pool(name="pslogT", bufs=2, space="PSUM") as pslogT,
            tc.tile_pool(name="psmain", bufs=2, space="PSUM") as psmain,
            tc.tile_pool(name="psS", bufs=2, space="PSUM") as psS,
        ):
            # ---- constants ----
            cwT = consts.tile([128, CCH, K], BF16, tag="cwT")
            nc.sync.dma_start(
                cwT[:], cwT_d.rearrange("(j i) k -> i j k", i=128))
            ident = consts.tile([128, 128], BF16, tag="ident")
            nc.sync.dma_start(ident[:], id_d)
            cen = consts.tile([K, C], F32, tag="cen")
            nc.sync.dma_start(cen[:], cen_d)

            def stage1a(s):
                """DMAs, first ssq chunks, mm1, logits copies."""
                xt = xt_pool.tile([128, PCHUNKS, C], F32R, tag="xt")
                nc.sync.dma_start(xt[:], xt_d[s])
                xna = xnat_pool.tile([128, CCH, P], BF16, tag="xna")
                nc.sync.dma_start(xna[:], x_d[s])

                nd = SSQ_DVE_CHUNKS
                ssqc = pvec_pool.tile([128, PCHUNKS], F32, tag="ssqc")
                stats = stats_pool.tile([128, nd, 6], F32, tag="stats")
                scr = sqscr_pool.tile([128, C], F32, tag="scr")
                for pj in range(nd // 2):
                    nc.vector.bn_stats(stats[:, pj, :], xt[:, pj, :].bitcast(F32))
                for pj in range(nd, nd + (PCHUNKS - nd) // 2):
                    nc.scalar.activation(
                        scr[:], xt[:, pj, :].bitcast(F32), AF.Square,
                        accum_out=ssqc[:, pj:pj + 1])

                # mm1: logits[K, P] (bf16, 4 stationary cwT chunks)
                logA = pslogits.tile([K, 450], F32, tag="logA")
                logB = pslogits.tile([K, 450], F32, tag="logB")
                for j in range(CCH):
                    nc.tensor.matmul(
                        logA[:], cwT[:, j, :], xna[:, j, 0:450],
                        start=(j == 0), stop=(j == CCH - 1))
                    nc.tensor.matmul(
                        logB[:], cwT[:, j, :], xna[:, j, 450:900],
                        start=(j == 0), stop=(j == CCH - 1))

                # logits -> sbuf bf16 (DVE+ACT halves); pad 900:1024 = 0
                logsb = work.tile([K, PPAD], BF16, tag="logsb")
                nc.vector.memset(logsb[:, P:PPAD], 0.0)
                nc.vector.tensor_copy(logsb[:, 0:450], logA[:])
                nc.scalar.copy(logsb[:, 450:900], logB[:])
                return s, xt, ssqc, stats, scr, logsb

            def stage1b(st):
                """Remaining ssq chunks, combine, logit transposes."""
                s, xt, ssqc, stats, scr, logsb = st
                nd = SSQ_DVE_CHUNKS
                for pj in range(nd // 2, nd):
                    nc.vector.bn_stats(stats[:, pj, :], xt[:, pj, :].bitcast(F32))
                for pj in range(nd + (PCHUNKS - nd) // 2, PCHUNKS):
                    nc.scalar.activation(
                        scr[:], xt[:, pj, :].bitcast(F32), AF.Square,
                        accum_out=ssqc[:, pj:pj + 1])
                # combine bn_stats -> ssq on DVE:
                # ssq = cvar_e + cvar_o + 256*(mean_e^2 + mean_o^2)
                sqm = pvec_pool.tile([128, nd, 2], F32, tag="sqm")
                nc.vector.tensor_mul(
                    sqm[:], stats[:, :, 1:6:3], stats[:, :, 1:6:3])
                vsum = pvec_pool.tile([128, nd], F32, tag="vsum")
                nc.vector.tensor_add(
                    vsum[:], stats[:, :, 2:3], stats[:, :, 5:6])
                msum = pvec_pool.tile([128, nd], F32, tag="msum")
                nc.vector.tensor_add(msum[:], sqm[:, :, 0], sqm[:, :, 1])
                m256 = pvec_pool.tile([128, nd], F32, tag="m256")
                nc.vector.tensor_scalar(
                    m256[:], msum[:], 256.0, None,
                    op0=mybir.AluOpType.mult)
                nc.vector.tensor_add(ssqc[:, 0:nd], vsum[:], m256[:])

                # transpose logits -> logT [pixel, K] (shared identity)
                logT = pslogT.tile([128, PCHUNKS * K], BF16, tag="logT")
                for pj in range(PCHUNKS):
                    nc.tensor.matmul(
                        logT[:, K * pj:K * (pj + 1)],
                        logsb[:, 128 * pj:128 * (pj + 1)],
                        ident[0:K, 0:K],
                        is_transpose=True,
                        skip_group_check=True,
                    )
                return s, xt, ssqc, logT

            def stage2a(st):
                """ln / +-0.5 exps on ACT, prescale on DVE."""
                s, xt, ssqc, logT = st
                lssq = pvec_pool.tile([128, PCHUNKS], F32, tag="lssq")
                nc.scalar.activation(lssq[:], ssqc[:], AF.Ln)
                invn = pvec_pool.tile([128, PCHUNKS], F32, tag="invn")
                nc.scalar.activation(invn[:], lssq[:], AF.Exp, scale=-0.5)
                # norm (padded to 9 cols; col 8 zeroed) for the S matmul rhs
                normc = pvec_pool.tile([128, PCHUNKS + 1], F32R, tag="normc")
                nc.vector.memset(normc[:, PCHUNKS:PCHUNKS + 1].bitcast(F32), 0.0)
                nc.scalar.activation(
                    normc[:, 0:PCHUNKS], lssq[:], AF.Exp, scale=0.5)
                lsc = work.tile([128, PCHUNKS, K], F32, tag="lsc")
                nc.vector.tensor_mul(
                    lsc[:],
                    logT[:].rearrange("i (c k) -> i c k", k=K),
                    invn[:].to_broadcast([128, PCHUNKS, K]))
                return s, xt, invn, normc, lsc

            def stage2b(st):
                """exp, softmax sums, aT."""
                s, xt, invn, normc, lsc = st
                e_sb = work.tile([128, PCHUNKS * K], F32, tag="esb")
                nc.scalar.activation(e_sb[:], lsc[:], AF.Exp)

                scol = pvec_pool.tile([128, PCHUNKS], F32, tag="scol")
                nc.vector.reduce_sum(
                    scol[:], e_sb[:].rearrange("i (c k) -> i c k", k=K),
                    axis=AX.X)
                invs = pvec_pool.tile([128, PCHUNKS], F32, tag="invs")
                nc.vector.reciprocal(invs[:], scol[:])
                tcol = pvec_pool.tile([128, PCHUNKS], F32, tag="tcol")
                nc.vector.tensor_mul(tcol[:], invs[:], invn[:])

                # aT = e * t (one DVE broadcast mul, f32r out for mm2)
                aT = work.tile([128, PCHUNKS, K], F32R, tag="aT")
                nc.vector.tensor_mul(
                    aT[:],
                    e_sb[:].rearrange("i (c k) -> i c k", k=K),
                    tcol[:].to_broadcast([128, PCHUNKS, K]))
                return s, xt, aT, normc

            def stage3(st):
                s, xt, aT, normc = st
                # mm2: main += aT.T @ xt ; S += aT.T @ norm (shared lhsT)
                main_ps = psmain.tile([K, C], F32, tag="main")
                S_ps = psS.tile([K, 2], F32, tag="Sps")
                for pj in range(PCHUNKS):
                    pw = 4 if pj == PCHUNKS - 1 else 128
                    nc.tensor.matmul(
                        main_ps[:], aT[0:pw, pj, :],
                        xt[0:pw, pj, :],
                        start=(pj == 0), stop=(pj == PCHUNKS - 1))
                    nc.tensor.matmul(
                        S_ps[:], aT[0:pw, pj, :],
                        normc[0:pw, pj:pj + 2],
                        start=(pj == 0), stop=(pj == PCHUNKS - 1))

                # final: out = main + (-S)*cen
                negS = pvec_pool.tile([K, 1], F32, tag="negS")
                nc.vector.tensor_scalar(
                    negS[:], S_ps[:, 0:1], -1.0, None,
                    op0=mybir.AluOpType.mult)
                tmp = outsb_pool.tile([K, C], F32, tag="tmp")
                nc.scalar.activation(tmp[:], cen[:], AF.Copy, scale=negS[:])
                out_sb = outsb_pool.tile([K, C], F32, tag="outsb")
                nc.vector.tensor_add(out_sb[:], main_ps[:], tmp[:])
                nc.sync.dma_start(out_d[s], out_sb[:])

            # skewed software pipeline:
            # iter k emits: stage3(k-1), 1a(k), 2a(k-1), 1b(k), 2b(k-1)
            samples = [s for _ in range(repeat) for s in range(S)]
            s1 = s2 = None
            for s in samples:
                a = stage1a(s)
                if s1 is not None:
                    mid = stage2a(s1)
                b = stage1b(a)
                if s1 is not None:
                    s2 = stage2b(mid)
                    stage3(s2)
                s1 = b
            stage3(stage2b(stage2a(s1)))

    try:
        nc.compile()
    finally:
        bacc.get_activation_tables = _orig_get_tables
    return nc


def _get_nc(repeat=1):
    key = ("nc", repeat)
    if key not in _cache:
        _cache[key] = _build_module(repeat)
    return _cache[key]


def build_in_maps(x, conv_w, centroids):
    import ml_dtypes

    bf16 = ml_dtypes.bfloat16
    x = np.asarray(x, dtype=np.float32)
    conv_w = np.asarray(conv_w, dtype=np.float32)
    centroids = np.asarray(centroids, dtype=np.float32)

    xs = x.reshape(N, C, P)
    # natural layout reordered to the SBUF tile shape [N, 128, CCH, P]
    x_bf = np.ascontiguousarray(
        xs.astype(bf16).reshape(N, CCH, 128, P).transpose(0, 2, 1, 3))
    # pixel-major, padded to PPAD with ones, reordered to [N, 128, PCH, C]
    xt_f = np.ones((N, PPAD, C), dtype=np.float32)
    xt_f[:, :P, :] = xs.transpose(0, 2, 1)
    xt_f = np.ascontiguousarray(
        xt_f.reshape(N, PCHUNKS, 128, C).transpose(0, 2, 1, 3))
    cwT = np.ascontiguousarray(conv_w.T).astype(bf16)   # [C, K]
    ident = np.eye(128, dtype=bf16)

    in_maps = []
    for core in range(NCORES):
        in_maps.append({
            "x": np.ascontiguousarray(x_bf[core * S:(core + 1) * S]),
            "xt": np.ascontiguousarray(xt_f[core * S:(core + 1) * S]),
            "cwT": cwT, "cen": centroids, "ident": ident,
        })
    return in_maps


def kernel(x, conv_w, centroids):
    from concourse.bass_utils import run_bass_kernel_spmd

    nc = _get_nc()
    in_maps = build_in_maps(x, conv_w, centroids)
    res = run_bass_kernel_spmd(nc, in_maps, core_ids=list(range(NCORES)))
    out = np.concatenate([r["vlad"] for r in res.results], axis=0)
    return out.reshape(N, K, C)
